# revision 7
# baseline (speedup 1.0000x reference)
"""Trainium2 Bass kernel for a causal self-attention block (GQA + per-head
RMS-norm + RoPE + learned q-gain), sharded over 8 NeuronCores.

Sharding: data-parallel over batch (B=2) as the outer axis x tensor-parallel
over head groups (4 groups of 4 query heads, each owning one KV head).
core = b*4 + g. Each core computes the full attention for its 4 heads and a
*partial* output projection (its 256 in-dims of Wproj); the host sums the 4
partials per batch element and transposes back.

Inside a core everything is computed in a transposed ("T") layout for the
attention matmuls: scores are built as S^T[k, q] = K @ Q^T so the PV matmul
can contract over keys on the partition axis; a row of ones appended to V
yields the softmax denominator for free.
"""

import math

import numpy as np

import concourse.bacc as bacc
import concourse.bass as bass
import concourse.tile as tile
from concourse import mybir
from concourse.bass import ts
from concourse.bass_utils import run_bass_kernel_spmd
from concourse.masks import make_identity

# Problem dims (hardcoded per contract).
B, S, D, H, KV, HD = 2, 2048, 1024, 16, 4, 64
NH = H // KV          # 4 query heads per core (one KV group)
GD = NH * HD          # 256 out-dims of Wq per group
P = 128               # partitions
NST = S // P          # 16 sequence tiles
JW = 512              # query-block width for attention
NJ = S // JW          # 4 query blocks
NC = 8                # cores
ROPE_BASE = 10000.0
RMS_EPS = 1.1920929e-07
F32 = mybir.dt.float32
F32R = mybir.dt.float32r
AXX = mybir.AxisListType.X
ACT = mybir.ActivationFunctionType


def _build_program():
    # Bacc (vs raw Bass) runs the TRN2 lowering passes: matmul waits moved to
    # ldweights, sync-wait splitting, act-table/library load insertion.
    nc = bacc.Bacc("TRN2", target_bir_lowering=False, debug=False)

    # f32r tensors hold ordinary f32 bits; the declaration lets the PE run
    # its full-rate fp32 path (4x faster than strict fp32 matmul).
    xT = nc.dram_tensor("xT", [D, S], F32R, kind="ExternalInput").ap()
    wqkv = nc.dram_tensor("wqkv", [D, GD + 2 * HD], F32R, kind="ExternalInput").ap()
    wp2 = nc.dram_tensor("wp2", [P, 2 * D], F32R, kind="ExternalInput").ap()
    cosn = nc.dram_tensor("cosn", [P, NST * 32], F32, kind="ExternalInput").ap()
    sinn = nc.dram_tensor("sinn", [P, NST * 32], F32, kind="ExternalInput").ap()
    masks = nc.dram_tensor("masks", [P, 4 * JW], F32, kind="ExternalInput").ap()
    qg8 = nc.dram_tensor("qg8", [1, NH], F32, kind="ExternalInput").ap()
    ypt = nc.dram_tensor("ypt", [D, S], F32, kind="ExternalOutput").ap()

    with tile.TileContext(nc) as tc:
        _body(tc, xT, wqkv, wp2, cosn, sinn, masks, qg8, ypt)
    nc.compile()
    return nc


def _body(tc, xT, wqkv, wp2, cosn, sinn, masks, qg8, ypt):
    nc = tc.nc
    NQKV = GD + 2 * HD  # 384

    with tc.tile_pool(name="consts", bufs=1) as consts:
        # Persistent SBUF state.
        wp_sb = consts.tile([P, 2, D], F32R, name="wp_sb")
        cos_sb = consts.tile([P, NST, 32], F32, name="cos_sb")
        sin_sb = consts.tile([P, NST, 32], F32, name="sin_sb")
        mask_sb = consts.tile([P, 4, JW], F32, name="mask_sb")
        qg8_sb = consts.tile([P, NH], F32, name="qg8_sb")
        ident = consts.tile([P, P], F32, name="ident")
        # qT/kT zero-padded to 128 partitions so attention matmuls run K=128
        # (no PE tiling-mode switches); rows 64-127 stay zero.
        qT_sb = consts.tile([P, NH, S], F32R, name="qT_sb")
        kT_sb = consts.tile([P, S], F32R, name="kT_sb")
        # V with a ones-column (65th) so PV accumulates softmax denominators.
        v_sb = consts.tile([P, NST, HD + 1], F32R, name="v_sb")
        # Normalized y^T, head pairs stacked on partitions for the out-proj.
        y_sb = consts.tile([P, 2, S], F32R, name="y_sb")
        # Selector matrix (row 64 all-ones) broadcasts the softmax denominator
        # over partitions via a plain K=128 matmul (no PE mode switch).
        sel64 = consts.tile([P, P], F32R, name="sel64")
        # Rotating staging rows for the reciprocal denominators: rows 0-63 and
        # 65-127 stay zero forever; row 64 is rewritten per use.
        bc0 = consts.tile([P, JW], F32R, name="bc0")
        bc1 = consts.tile([P, JW], F32R, name="bc1")

        nc.sync.dma_start(out=wp_sb, in_=wp2.rearrange("p (c m) -> p c m", c=2))
        nc.sync.dma_start(out=cos_sb, in_=cosn.rearrange("p (t f) -> p t f", f=32))
        nc.sync.dma_start(out=sin_sb, in_=sinn.rearrange("p (t f) -> p t f", f=32))
        nc.sync.dma_start(out=mask_sb, in_=masks.rearrange("p (m c) -> p m c", c=JW))
        nc.gpsimd.dma_start(out=qg8_sb, in_=qg8.to_broadcast([P, NH]))
        make_identity(nc, ident)
        # f32r tiles can't be memset directly (ISA); fill via f32 -> f32r
        # broadcast copies, which are legal rounding producers.
        z1 = consts.tile([P, 1], F32, name="z1")
        o1 = consts.tile([P, 1], F32, name="o1")
        nc.vector.memset(z1, 0.0)
        nc.vector.memset(o1, 1.0)
        nc.vector.tensor_copy(
            v_sb[:, :, HD : HD + 1], o1[:, None, :].broadcast_to([P, NST, 1])
        )
        nc.vector.tensor_copy(
            qT_sb[HD:P, :, :], z1[HD:P, :][:, None, :].broadcast_to([HD, NH, S])
        )
        nc.vector.tensor_copy(kT_sb[HD:P, :], z1[HD:P, :].broadcast_to([HD, S]))
        nc.vector.tensor_copy(sel64, z1.broadcast_to([P, P]))
        nc.vector.tensor_copy(
            sel64[HD : HD + 1, :], o1[HD : HD + 1, :].broadcast_to([1, P])
        )
        nc.vector.tensor_copy(bc0, z1.broadcast_to([P, JW]))
        nc.vector.tensor_copy(bc1, z1.broadcast_to([P, JW]))

        # ---------------- Phase 1: QKV proj + RMS + RoPE + transposes -----
        with tc.tile_pool(name="ph1c", bufs=1) as ph1c:
            xT_sb = ph1c.tile([P, 8, S], F32R, name="xT_sb")
            w_sb = ph1c.tile([P, 8, NQKV], F32R, name="w_sb")
            xTr = xT.rearrange("(c p) s -> p c s", p=P)
            for c in range(8):
                nc.sync.dma_start(out=xT_sb[:, c, :], in_=xTr[:, c, :])
            nc.sync.dma_start(out=w_sb, in_=wqkv.rearrange("(c p) n -> p c n", p=P))

            with (
                tc.tile_pool(name="p1w", bufs=3) as work,
                tc.tile_pool(name="p1ps", bufs=3, space="PSUM") as psP,
                tc.tile_pool(name="p1pt", bufs=4, space="PSUM") as psT,
            ):
                for i in range(NST):
                    qkv_ps = psP.tile([P, NQKV], F32, name=f"qkv_ps{i}", tag="qkv")
                    for c in range(8):
                        nc.tensor.matmul(
                            qkv_ps,
                            lhsT=xT_sb[:, c, ts(i, P)],
                            rhs=w_sb[:, c, :],
                            start=(c == 0),
                            stop=(c == 7),
                        )
                    # V tile straight out of PSUM.
                    nc.scalar.copy(v_sb[:, i, 0:HD], qkv_ps[:, GD + HD : NQKV])

                    # RMS statistics for the 4 q heads at once, then k.
                    sq4 = work.tile([P, GD], F32, name=f"sq4_{i}", tag="sq4")
                    nc.scalar.square(sq4, qkv_ps[:, 0:GD])
                    qsc = work.tile([P, NH], F32, name=f"qsc_{i}", tag="qsc")
                    nc.vector.reduce_sum(
                        qsc, sq4.rearrange("p (h d) -> p h d", d=HD), axis=AXX
                    )
                    sqk = work.tile([P, HD], F32, name=f"sqk_{i}", tag="sqk")
                    nc.scalar.square(sqk, qkv_ps[:, GD : GD + HD])
                    ksc = work.tile([P, 1], F32, name=f"ksc_{i}", tag="ksc")
                    nc.vector.reduce_sum(ksc, sqk, axis=AXX)

                    def rsqrt_(m_out, nhd, i=i):
                        # m_out holds sum(x^2); converts to rsqrt(mean+eps)
                        # with one Newton step (ACT sqrt table is low-ULP).
                        m = work.tile([P, nhd], F32, name=f"m{nhd}_{i}", tag=f"m{nhd}")
                        nc.vector.tensor_scalar(
                            out=m, in0=m_out, scalar1=1.0 / HD, scalar2=RMS_EPS,
                            op0=mybir.AluOpType.mult, op1=mybir.AluOpType.add,
                        )
                        nc.scalar.activation(m_out, m, ACT.Sqrt)
                        nc.vector.reciprocal(m_out, m_out)  # r0 ~ rsqrt
                        t = work.tile([P, nhd], F32, name=f"t{nhd}_{i}", tag=f"t{nhd}")
                        nc.vector.tensor_mul(t, m, m_out)
                        nc.vector.tensor_mul(t, t, m_out)
                        nc.vector.tensor_scalar(
                            out=t, in0=t, scalar1=-0.5, scalar2=1.5,
                            op0=mybir.AluOpType.mult, op1=mybir.AluOpType.add,
                        )
                        nc.vector.tensor_mul(m_out, m_out, t)

                    rsqrt_(qsc, NH)
                    nc.vector.tensor_mul(qsc, qsc, qg8_sb)  # fold gain/8 into q
                    rsqrt_(ksc, 1)

                    # RoPE on q (4 heads batched) and k, then scale.
                    q3 = qkv_ps[:, 0:GD].rearrange("p (h d) -> p h d", d=HD)
                    cosb = cos_sb[:, i, :][:, None, :].broadcast_to([P, NH, 32])
                    sinb = sin_sb[:, i, :][:, None, :].broadcast_to([P, NH, 32])
                    qrot = work.tile([P, NH, HD], F32, name=f"qrot_{i}", tag="qrot")
                    ta = work.tile([P, NH, 32], F32, name=f"ta_{i}", tag="ta")
                    tb = work.tile([P, NH, 32], F32, name=f"tb_{i}", tag="tb")
                    tc2 = work.tile([P, NH, 32], F32, name=f"tc_{i}", tag="tc")
                    td = work.tile([P, NH, 32], F32, name=f"td_{i}", tag="td")
                    nc.vector.tensor_mul(ta, q3[:, :, 0:32], cosb)
                    nc.vector.tensor_mul(tb, q3[:, :, 32:HD], sinb)
                    nc.vector.tensor_add(qrot[:, :, 0:32], ta, tb)
                    nc.vector.tensor_mul(tc2, q3[:, :, 32:HD], cosb)
                    nc.vector.tensor_mul(td, q3[:, :, 0:32], sinb)
                    nc.vector.tensor_sub(qrot[:, :, 32:HD], tc2, td)
                    nc.vector.tensor_mul(
                        qrot, qrot, qsc[:, :, None].broadcast_to([P, NH, HD])
                    )

                    kc = qkv_ps[:, GD : GD + HD]
                    krot = work.tile([P, HD], F32, name=f"krot_{i}", tag="krot")
                    ka = work.tile([P, 32], F32, name=f"ka_{i}", tag="ka")
                    kb = work.tile([P, 32], F32, name=f"kb_{i}", tag="kb")
                    kc2 = work.tile([P, 32], F32, name=f"kc2_{i}", tag="kc2")
                    kd = work.tile([P, 32], F32, name=f"kd_{i}", tag="kd")
                    nc.vector.tensor_mul(ka, kc[:, 0:32], cos_sb[:, i, :])
                    nc.vector.tensor_mul(kb, kc[:, 32:HD], sin_sb[:, i, :])
                    nc.vector.tensor_add(krot[:, 0:32], ka, kb)
                    nc.vector.tensor_mul(kc2, kc[:, 32:HD], cos_sb[:, i, :])
                    nc.vector.tensor_mul(kd, kc[:, 0:32], sin_sb[:, i, :])
                    nc.vector.tensor_sub(krot[:, 32:HD], kc2, kd)
                    nc.vector.tensor_scalar_mul(krot, in0=krot, scalar1=ksc)

                    # Transpose to [d, s] layouts (PE transpose via identity).
                    for h in range(NH):
                        trq = psT.tile([HD, P], F32, name=f"trq_{i}_{h}", tag="tr")
                        nc.tensor.transpose(trq, qrot[:, h, :], ident)
                        nc.vector.tensor_copy(qT_sb[0:HD, h, ts(i, P)], trq)
                    trk = psT.tile([HD, P], F32, name=f"trk_{i}", tag="tr")
                    nc.tensor.transpose(trk, krot, ident)
                    nc.vector.tensor_copy(kT_sb[0:HD, ts(i, P)], trk)

        # ---------------- Phase 2: attention --------------------------------
        with (
            tc.tile_pool(name="p2w", bufs=3) as workp,
            tc.tile_pool(name="p2s", bufs=2, space="PSUM") as psS,
            tc.tile_pool(name="p2y", bufs=2, space="PSUM") as psY,
            tc.tile_pool(name="p2b", bufs=2, space="PSUM") as psB,
        ):
            for h in range(NH):
                for j in range(NJ):
                    nt = 4 * (j + 1)  # valid k-tiles for this q block
                    y_ps = psY.tile([HD + 1, JW], F32, name=f"y_ps{h}_{j}", tag="y")
                    qh = qT_sb[:, h, ts(j, JW)]
                    for cc in range(nt // 2):
                        st = psS.tile([P, 2 * JW], F32, name=f"st{h}_{j}_{cc}", tag="st")
                        for u in range(2):
                            t = 2 * cc + u
                            nc.tensor.matmul(
                                st[:, ts(u, JW)],
                                lhsT=kT_sb[:, ts(t, P)],
                                rhs=qh,
                                start=True,
                                stop=True,
                            )
                        p_sb = workp.tile([P, 2 * JW], F32R, name=f"p{h}_{j}_{cc}", tag="p")
                        nc.scalar.activation(p_sb, st, ACT.Exp)
                        for u in range(2):
                            m = 2 * cc + u - 4 * j
                            if m >= 0:  # diagonal tile: zero the future keys
                                nc.vector.tensor_mul(
                                    p_sb[:, ts(u, JW)], p_sb[:, ts(u, JW)],
                                    mask_sb[:, m, :],
                                )
                        for u in range(2):
                            t = 2 * cc + u
                            nc.tensor.matmul(
                                y_ps,
                                lhsT=v_sb[:, t, :],
                                rhs=p_sb[:, ts(u, JW)],
                                start=(t == 0),
                                stop=(t == nt - 1),
                            )
                    # Softmax normalization: row HD of y_ps is the denom.
                    bc = (bc0, bc1)[(h * NJ + j) % 2]
                    with nc.allow_low_precision(reason="f32r denom row"):
                        nc.vector.reciprocal(
                            bc[HD : HD + 1, :], y_ps[HD : HD + 1, :]
                        )
                    bcp = psB.tile([P, JW], F32, name=f"bcp{h}_{j}", tag="bcp")
                    nc.tensor.matmul(
                        bcp, lhsT=sel64, rhs=bc, start=True, stop=True
                    )
                    bcs = workp.tile([HD, JW], F32, name=f"bcs{h}_{j}", tag="bcs")
                    nc.vector.tensor_copy(bcs, bcp[0:HD, :])
                    if h % 2 == 0:
                        nc.vector.tensor_mul(
                            y_sb[0:HD, h // 2, ts(j, JW)], y_ps[0:HD, :], bcs
                        )
                    else:
                        ytmp = workp.tile([HD, JW], F32R, name=f"yt{h}_{j}", tag="ytmp")
                        nc.vector.tensor_mul(ytmp, y_ps[0:HD, :], bcs)
                        nc.sync.dma_start(
                            out=y_sb[HD:P, h // 2, ts(j, JW)], in_=ytmp
                        )

        # ---------------- Phase 3: output projection (partial) --------------
        with (
            tc.tile_pool(name="p3w", bufs=4) as worko,
            tc.tile_pool(name="p3ps", bufs=4, space="PSUM") as psO,
        ):
            for m in range(D // P):
                for j in range(NJ):
                    op_ps = psO.tile([P, JW], F32, name=f"op{m}_{j}", tag="op")
                    for c in range(2):
                        nc.tensor.matmul(
                            op_ps,
                            lhsT=wp_sb[:, c, ts(m, P)],
                            rhs=y_sb[:, c, ts(j, JW)],
                            start=(c == 0),
                            stop=(c == 1),
                        )
                    o_sb = worko.tile([P, JW], F32, name=f"o{m}_{j}", tag="o")
                    nc.vector.tensor_copy(o_sb, op_ps)
                    nc.sync.dma_start(out=ypt[ts(m, P), ts(j, JW)], in_=o_sb)


_PROG = None


def _get_program():
    global _PROG
    if _PROG is None:
        _PROG = _build_program()
    return _PROG


def _host_tables():
    inv_freq = (1.0 / (ROPE_BASE ** (np.arange(0, HD, 2, dtype=np.float32) / HD))).astype(
        np.float32
    )
    t = np.arange(S, dtype=np.float32)
    freqs = t[:, None] * inv_freq[None, :]  # [S, 32]
    cosf = np.cos(freqs).astype(np.float32)
    sinf = np.sin(freqs).astype(np.float32)
    # natural per-s-tile layout: [p, tile, freq]
    cosn = np.ascontiguousarray(
        cosf.reshape(NST, P, 32).transpose(1, 0, 2).reshape(P, NST * 32)
    )
    sinn = np.ascontiguousarray(
        sinf.reshape(NST, P, 32).transpose(1, 0, 2).reshape(P, NST * 32)
    )
    p_idx = np.arange(P)[:, None]
    c_idx = np.arange(JW)[None, :]
    mlist = [(c_idx >= m * P + p_idx).astype(np.float32) for m in range(4)]
    masks = np.ascontiguousarray(np.concatenate(mlist, axis=1))  # [128, 2048]
    return cosn, sinn, masks


def _in_maps(x, Wq, Wk, Wv, Wproj, q_gain):
    cosn, sinn, masks = _host_tables()
    maps = []
    for core in range(NC):
        b, g = divmod(core, KV)
        xT = np.ascontiguousarray(x[b].T)  # [D, S]
        wqkv = np.ascontiguousarray(
            np.concatenate(
                [
                    Wq[g * GD : (g + 1) * GD].T,
                    Wk[g * HD : (g + 1) * HD].T,
                    Wv[g * HD : (g + 1) * HD].T,
                ],
                axis=1,
            )
        )  # [D, 384]
        wsl = Wproj[:, g * GD : (g + 1) * GD].T.reshape(NH, HD, D)  # [head, d, m]
        wp2 = np.ascontiguousarray(
            np.stack(
                [
                    np.concatenate([wsl[0], wsl[1]], axis=0),
                    np.concatenate([wsl[2], wsl[3]], axis=0),
                ],
                axis=1,
            ).reshape(P, 2 * D)
        )
        qg8 = np.ascontiguousarray(
            (q_gain[g * NH : (g + 1) * NH] / 8.0).astype(np.float32).reshape(1, NH)
        )
        maps.append(
            {
                "xT": xT,
                "wqkv": wqkv,
                "wp2": wp2,
                "cosn": cosn,
                "sinn": sinn,
                "masks": masks,
                "qg8": qg8,
            }
        )
    return maps


def kernel(x, Wq, Wk, Wv, Wproj, q_gain, _collect=None):
    x = np.asarray(x, dtype=np.float32)
    Wq = np.asarray(Wq, dtype=np.float32)
    Wk = np.asarray(Wk, dtype=np.float32)
    Wv = np.asarray(Wv, dtype=np.float32)
    Wproj = np.asarray(Wproj, dtype=np.float32)
    q_gain = np.asarray(q_gain, dtype=np.float32)

    nc = _get_program()
    maps = _in_maps(x, Wq, Wk, Wv, Wproj, q_gain)
    res = run_bass_kernel_spmd(nc, maps, core_ids=list(range(NC)))
    if _collect is not None:
        _collect.append(res)

    out = np.zeros((B, S, D), dtype=np.float64)
    for core in range(NC):
        b, _ = divmod(core, KV)
        out[b] += res.results[core]["ypt"].T.astype(np.float64)
    return out.astype(np.float32)


# revision 8
# speedup vs baseline: 1.7347x; 1.7347x over previous
"""Trainium2 Bass kernel for a causal self-attention block (GQA + per-head
RMS-norm + RoPE + learned q-gain), sharded over 8 NeuronCores.

Sharding: data-parallel over batch (B=2) as the outer axis x tensor-parallel
over head groups (4 groups of 4 query heads, each owning one KV head).
core = b*4 + g. Each core computes the full attention for its 4 heads and a
*partial* output projection (its 256 in-dims of Wproj); the host sums the 4
partials per batch element and transposes back.

Inside a core everything is computed in a transposed ("T") layout for the
attention matmuls: scores are built as S^T[k, q] = K @ Q^T so the PV matmul
can contract over keys on the partition axis; a row of ones appended to V
yields the softmax denominator for free.
"""

import math

import numpy as np

import concourse.bacc as bacc
import concourse.bass as bass
import concourse.tile as tile
from concourse import mybir
from concourse.bass import ts
from concourse.bass_utils import run_bass_kernel_spmd
from concourse.masks import make_identity

# Problem dims (hardcoded per contract).
B, S, D, H, KV, HD = 2, 2048, 1024, 16, 4, 64
NH = H // KV          # 4 query heads per core (one KV group)
GD = NH * HD          # 256 out-dims of Wq per group
P = 128               # partitions
NST = S // P          # 16 sequence tiles
JW = 512              # query-block width for attention
NJ = S // JW          # 4 query blocks
NC = 8                # cores
ROPE_BASE = 10000.0
RMS_EPS = 1.1920929e-07
F32 = mybir.dt.float32
F32R = mybir.dt.float32r
AXX = mybir.AxisListType.X
ACT = mybir.ActivationFunctionType


def _build_program(reps=1):
    # Bacc (vs raw Bass) runs the TRN2 lowering passes: matmul waits moved to
    # ldweights, sync-wait splitting, act-table/library load insertion.
    # reps>1 replicates the whole body for slope-based benchmarking.
    nc = bacc.Bacc("TRN2", target_bir_lowering=False, debug=False)

    # f32r tensors hold ordinary f32 bits; the declaration lets the PE run
    # its full-rate fp32 path (4x faster than strict fp32 matmul).
    xT = nc.dram_tensor("xT", [D, S], F32R, kind="ExternalInput").ap()
    wqkv = nc.dram_tensor("wqkv", [D, GD + 2 * HD], F32R, kind="ExternalInput").ap()
    wp2 = nc.dram_tensor("wp2", [P, 2 * D], F32R, kind="ExternalInput").ap()
    cosn = nc.dram_tensor("cosn", [P, NST * 32], F32, kind="ExternalInput").ap()
    sinn = nc.dram_tensor("sinn", [P, NST * 32], F32, kind="ExternalInput").ap()
    masks = nc.dram_tensor("masks", [P, 4 * JW], F32, kind="ExternalInput").ap()
    qg8 = nc.dram_tensor("qg8", [1, NH], F32, kind="ExternalInput").ap()
    ypt = nc.dram_tensor("ypt", [D, S], F32, kind="ExternalOutput").ap()

    with tile.TileContext(nc) as tc:
        for _ in range(reps):
            _body(tc, xT, wqkv, wp2, cosn, sinn, masks, qg8, ypt)
    nc.compile()
    return nc


def _body(tc, xT, wqkv, wp2, cosn, sinn, masks, qg8, ypt):
    nc = tc.nc
    NQKV = GD + 2 * HD  # 384

    with tc.tile_pool(name="consts", bufs=1) as consts:
        # Persistent SBUF state.
        wp_sb = consts.tile([P, 2, D], F32R, name="wp_sb")
        cos_sb = consts.tile([P, NST, 32], F32, name="cos_sb")
        sin_sb = consts.tile([P, NST, 32], F32, name="sin_sb")
        mask_sb = consts.tile([P, 4, JW], F32, name="mask_sb")
        qg8_sb = consts.tile([P, NH], F32, name="qg8_sb")
        ident = consts.tile([P, P], F32, name="ident")
        # qT/kT zero-padded to 128 partitions so attention matmuls run K=128
        # (no PE tiling-mode switches); rows 64-127 stay zero.
        qT_sb = consts.tile([P, NH, S], F32R, name="qT_sb")
        kT_sb = consts.tile([P, S], F32R, name="kT_sb")
        # V with a ones-column (65th) so PV accumulates softmax denominators.
        v_sb = consts.tile([P, NST, HD + 1], F32R, name="v_sb")
        # Normalized y^T, head pairs stacked on partitions for the out-proj.
        y_sb = consts.tile([P, 2, S], F32R, name="y_sb")
        # Selector matrix (row 64 all-ones) broadcasts the softmax denominator
        # over partitions via a plain K=128 matmul (no PE mode switch).
        sel64 = consts.tile([P, P], F32R, name="sel64")
        # Rotating staging rows for the reciprocal denominators: rows 0-63 and
        # 65-127 stay zero forever; row 64 is rewritten per use.
        bc0 = consts.tile([P, JW], F32R, name="bc0")
        bc1 = consts.tile([P, JW], F32R, name="bc1")

        nc.sync.dma_start(out=wp_sb, in_=wp2.rearrange("p (c m) -> p c m", c=2))
        nc.sync.dma_start(out=cos_sb, in_=cosn.rearrange("p (t f) -> p t f", f=32))
        nc.sync.dma_start(out=sin_sb, in_=sinn.rearrange("p (t f) -> p t f", f=32))
        nc.sync.dma_start(out=mask_sb, in_=masks.rearrange("p (m c) -> p m c", c=JW))
        nc.gpsimd.dma_start(out=qg8_sb, in_=qg8.to_broadcast([P, NH]))
        make_identity(nc, ident)
        # f32r tiles can't be memset directly (ISA); fill via f32 -> f32r
        # broadcast copies, which are legal rounding producers.
        z1 = consts.tile([P, 1], F32, name="z1")
        o1 = consts.tile([P, 1], F32, name="o1")
        nc.vector.memset(z1, 0.0)
        nc.vector.memset(o1, 1.0)
        nc.vector.tensor_copy(
            v_sb[:, :, HD : HD + 1], o1[:, None, :].broadcast_to([P, NST, 1])
        )
        nc.vector.tensor_copy(
            qT_sb[HD:P, :, :], z1[HD:P, :][:, None, :].broadcast_to([HD, NH, S])
        )
        nc.vector.tensor_copy(kT_sb[HD:P, :], z1[HD:P, :].broadcast_to([HD, S]))
        nc.vector.tensor_copy(sel64, z1.broadcast_to([P, P]))
        nc.vector.tensor_copy(
            sel64[HD : HD + 1, :], o1[HD : HD + 1, :].broadcast_to([1, P])
        )
        nc.vector.tensor_copy(bc0, z1.broadcast_to([P, JW]))
        nc.vector.tensor_copy(bc1, z1.broadcast_to([P, JW]))

        # ---------------- Phase 1: QKV proj + RMS + RoPE + transposes -----
        with tc.tile_pool(name="ph1c", bufs=1) as ph1c:
            xT_sb = ph1c.tile([P, 8, S], F32R, name="xT_sb")
            w_sb = ph1c.tile([P, 8, NQKV], F32R, name="w_sb")
            xTr = xT.rearrange("(c p) s -> p c s", p=P)
            for c in range(8):
                nc.sync.dma_start(out=xT_sb[:, c, :], in_=xTr[:, c, :])
            nc.sync.dma_start(out=w_sb, in_=wqkv.rearrange("(c p) n -> p c n", p=P))

            with (
                tc.tile_pool(name="p1w", bufs=3) as work,
                tc.tile_pool(name="p1ps", bufs=3, space="PSUM") as psP,
                tc.tile_pool(name="p1pt", bufs=4, space="PSUM") as psT,
            ):
                for i in range(NST):
                    qkv_ps = psP.tile([P, NQKV], F32, name=f"qkv_ps{i}", tag="qkv")
                    for c in range(8):
                        nc.tensor.matmul(
                            qkv_ps,
                            lhsT=xT_sb[:, c, ts(i, P)],
                            rhs=w_sb[:, c, :],
                            start=(c == 0),
                            stop=(c == 7),
                        )
                    # V tile straight out of PSUM.
                    nc.scalar.copy(v_sb[:, i, 0:HD], qkv_ps[:, GD + HD : NQKV])

                    # RMS statistics for the 4 q heads at once, then k.
                    sq4 = work.tile([P, GD], F32, name=f"sq4_{i}", tag="sq4")
                    nc.scalar.square(sq4, qkv_ps[:, 0:GD])
                    qsc = work.tile([P, NH], F32, name=f"qsc_{i}", tag="qsc")
                    nc.vector.reduce_sum(
                        qsc, sq4.rearrange("p (h d) -> p h d", d=HD), axis=AXX
                    )
                    sqk = work.tile([P, HD], F32, name=f"sqk_{i}", tag="sqk")
                    nc.scalar.square(sqk, qkv_ps[:, GD : GD + HD])
                    ksc = work.tile([P, 1], F32, name=f"ksc_{i}", tag="ksc")
                    nc.vector.reduce_sum(ksc, sqk, axis=AXX)

                    def rsqrt_(m_out, nhd, i=i):
                        # m_out holds sum(x^2); converts to rsqrt(mean+eps)
                        # with one Newton step (ACT sqrt table is low-ULP).
                        m = work.tile([P, nhd], F32, name=f"m{nhd}_{i}", tag=f"m{nhd}")
                        nc.vector.tensor_scalar(
                            out=m, in0=m_out, scalar1=1.0 / HD, scalar2=RMS_EPS,
                            op0=mybir.AluOpType.mult, op1=mybir.AluOpType.add,
                        )
                        nc.scalar.activation(m_out, m, ACT.Sqrt)
                        nc.vector.reciprocal(m_out, m_out)  # r0 ~ rsqrt
                        t = work.tile([P, nhd], F32, name=f"t{nhd}_{i}", tag=f"t{nhd}")
                        nc.vector.tensor_mul(t, m, m_out)
                        nc.vector.tensor_mul(t, t, m_out)
                        nc.vector.tensor_scalar(
                            out=t, in0=t, scalar1=-0.5, scalar2=1.5,
                            op0=mybir.AluOpType.mult, op1=mybir.AluOpType.add,
                        )
                        nc.vector.tensor_mul(m_out, m_out, t)

                    rsqrt_(qsc, NH)
                    nc.vector.tensor_mul(qsc, qsc, qg8_sb)  # fold gain/8 into q
                    rsqrt_(ksc, 1)

                    # RoPE on q (4 heads batched) and k, then scale.
                    q3 = qkv_ps[:, 0:GD].rearrange("p (h d) -> p h d", d=HD)
                    cosb = cos_sb[:, i, :][:, None, :].broadcast_to([P, NH, 32])
                    sinb = sin_sb[:, i, :][:, None, :].broadcast_to([P, NH, 32])
                    qrot = work.tile([P, NH, HD], F32, name=f"qrot_{i}", tag="qrot")
                    ta = work.tile([P, NH, 32], F32, name=f"ta_{i}", tag="ta")
                    tb = work.tile([P, NH, 32], F32, name=f"tb_{i}", tag="tb")
                    tc2 = work.tile([P, NH, 32], F32, name=f"tc_{i}", tag="tc")
                    td = work.tile([P, NH, 32], F32, name=f"td_{i}", tag="td")
                    nc.vector.tensor_mul(ta, q3[:, :, 0:32], cosb)
                    nc.vector.tensor_mul(tb, q3[:, :, 32:HD], sinb)
                    nc.vector.tensor_add(qrot[:, :, 0:32], ta, tb)
                    nc.vector.tensor_mul(tc2, q3[:, :, 32:HD], cosb)
                    nc.vector.tensor_mul(td, q3[:, :, 0:32], sinb)
                    nc.vector.tensor_sub(qrot[:, :, 32:HD], tc2, td)
                    nc.vector.tensor_mul(
                        qrot, qrot, qsc[:, :, None].broadcast_to([P, NH, HD])
                    )

                    kc = qkv_ps[:, GD : GD + HD]
                    krot = work.tile([P, HD], F32, name=f"krot_{i}", tag="krot")
                    ka = work.tile([P, 32], F32, name=f"ka_{i}", tag="ka")
                    kb = work.tile([P, 32], F32, name=f"kb_{i}", tag="kb")
                    kc2 = work.tile([P, 32], F32, name=f"kc2_{i}", tag="kc2")
                    kd = work.tile([P, 32], F32, name=f"kd_{i}", tag="kd")
                    nc.vector.tensor_mul(ka, kc[:, 0:32], cos_sb[:, i, :])
                    nc.vector.tensor_mul(kb, kc[:, 32:HD], sin_sb[:, i, :])
                    nc.vector.tensor_add(krot[:, 0:32], ka, kb)
                    nc.vector.tensor_mul(kc2, kc[:, 32:HD], cos_sb[:, i, :])
                    nc.vector.tensor_mul(kd, kc[:, 0:32], sin_sb[:, i, :])
                    nc.vector.tensor_sub(krot[:, 32:HD], kc2, kd)
                    nc.vector.tensor_scalar_mul(krot, in0=krot, scalar1=ksc)

                    # Transpose to [d, s] layouts (PE transpose via identity).
                    for h in range(NH):
                        trq = psT.tile([HD, P], F32, name=f"trq_{i}_{h}", tag="tr")
                        nc.tensor.transpose(trq, qrot[:, h, :], ident)
                        nc.vector.tensor_copy(qT_sb[0:HD, h, ts(i, P)], trq)
                    trk = psT.tile([HD, P], F32, name=f"trk_{i}", tag="tr")
                    nc.tensor.transpose(trk, krot, ident)
                    nc.vector.tensor_copy(kT_sb[0:HD, ts(i, P)], trk)

        # ---------------- Phase 2: attention --------------------------------
        with (
            tc.tile_pool(name="p2w", bufs=3) as workp,
            tc.tile_pool(name="p2s", bufs=2, space="PSUM") as psS,
            tc.tile_pool(name="p2y", bufs=2, space="PSUM") as psY,
            tc.tile_pool(name="p2b", bufs=2, space="PSUM") as psB,
        ):
            for h in range(NH):
                for j in range(NJ):
                    nt = 4 * (j + 1)  # valid k-tiles for this q block
                    y_ps = psY.tile([HD + 1, JW], F32, name=f"y_ps{h}_{j}", tag="y")
                    qh = qT_sb[:, h, ts(j, JW)]
                    for cc in range(nt // 2):
                        st = psS.tile([P, 2 * JW], F32, name=f"st{h}_{j}_{cc}", tag="st")
                        for u in range(2):
                            t = 2 * cc + u
                            nc.tensor.matmul(
                                st[:, ts(u, JW)],
                                lhsT=kT_sb[:, ts(t, P)],
                                rhs=qh,
                                start=True,
                                stop=True,
                            )
                        p_sb = workp.tile([P, 2 * JW], F32R, name=f"p{h}_{j}_{cc}", tag="p")
                        nc.scalar.activation(p_sb, st, ACT.Exp)
                        for u in range(2):
                            m = 2 * cc + u - 4 * j
                            if m >= 0:  # diagonal tile: zero the future keys
                                nc.vector.tensor_mul(
                                    p_sb[:, ts(u, JW)], p_sb[:, ts(u, JW)],
                                    mask_sb[:, m, :],
                                )
                        for u in range(2):
                            t = 2 * cc + u
                            nc.tensor.matmul(
                                y_ps,
                                lhsT=v_sb[:, t, :],
                                rhs=p_sb[:, ts(u, JW)],
                                start=(t == 0),
                                stop=(t == nt - 1),
                            )
                    # Softmax normalization: row HD of y_ps is the denom.
                    bc = (bc0, bc1)[(h * NJ + j) % 2]
                    with nc.allow_low_precision(reason="f32r denom row"):
                        nc.vector.reciprocal(
                            bc[HD : HD + 1, :], y_ps[HD : HD + 1, :]
                        )
                    bcp = psB.tile([P, JW], F32, name=f"bcp{h}_{j}", tag="bcp")
                    nc.tensor.matmul(
                        bcp, lhsT=sel64, rhs=bc, start=True, stop=True
                    )
                    bcs = workp.tile([HD, JW], F32, name=f"bcs{h}_{j}", tag="bcs")
                    nc.vector.tensor_copy(bcs, bcp[0:HD, :])
                    if h % 2 == 0:
                        nc.vector.tensor_mul(
                            y_sb[0:HD, h // 2, ts(j, JW)], y_ps[0:HD, :], bcs
                        )
                    else:
                        ytmp = workp.tile([HD, JW], F32R, name=f"yt{h}_{j}", tag="ytmp")
                        nc.vector.tensor_mul(ytmp, y_ps[0:HD, :], bcs)
                        nc.sync.dma_start(
                            out=y_sb[HD:P, h // 2, ts(j, JW)], in_=ytmp
                        )

        # ---------------- Phase 3: output projection (partial) --------------
        with (
            tc.tile_pool(name="p3w", bufs=4) as worko,
            tc.tile_pool(name="p3ps", bufs=4, space="PSUM") as psO,
        ):
            for m in range(D // P):
                for j in range(NJ):
                    op_ps = psO.tile([P, JW], F32, name=f"op{m}_{j}", tag="op")
                    for c in range(2):
                        nc.tensor.matmul(
                            op_ps,
                            lhsT=wp_sb[:, c, ts(m, P)],
                            rhs=y_sb[:, c, ts(j, JW)],
                            start=(c == 0),
                            stop=(c == 1),
                        )
                    o_sb = worko.tile([P, JW], F32, name=f"o{m}_{j}", tag="o")
                    nc.vector.tensor_copy(o_sb, op_ps)
                    nc.sync.dma_start(out=ypt[ts(m, P), ts(j, JW)], in_=o_sb)


_PROG = None


def _get_program():
    global _PROG
    if _PROG is None:
        _PROG = _build_program()
    return _PROG


def _host_tables():
    inv_freq = (1.0 / (ROPE_BASE ** (np.arange(0, HD, 2, dtype=np.float32) / HD))).astype(
        np.float32
    )
    t = np.arange(S, dtype=np.float32)
    freqs = t[:, None] * inv_freq[None, :]  # [S, 32]
    cosf = np.cos(freqs).astype(np.float32)
    sinf = np.sin(freqs).astype(np.float32)
    # natural per-s-tile layout: [p, tile, freq]
    cosn = np.ascontiguousarray(
        cosf.reshape(NST, P, 32).transpose(1, 0, 2).reshape(P, NST * 32)
    )
    sinn = np.ascontiguousarray(
        sinf.reshape(NST, P, 32).transpose(1, 0, 2).reshape(P, NST * 32)
    )
    p_idx = np.arange(P)[:, None]
    c_idx = np.arange(JW)[None, :]
    mlist = [(c_idx >= m * P + p_idx).astype(np.float32) for m in range(4)]
    masks = np.ascontiguousarray(np.concatenate(mlist, axis=1))  # [128, 2048]
    return cosn, sinn, masks


def _in_maps(x, Wq, Wk, Wv, Wproj, q_gain):
    cosn, sinn, masks = _host_tables()
    maps = []
    for core in range(NC):
        b, g = divmod(core, KV)
        xT = np.ascontiguousarray(x[b].T)  # [D, S]
        wqkv = np.ascontiguousarray(
            np.concatenate(
                [
                    Wq[g * GD : (g + 1) * GD].T,
                    Wk[g * HD : (g + 1) * HD].T,
                    Wv[g * HD : (g + 1) * HD].T,
                ],
                axis=1,
            )
        )  # [D, 384]
        wsl = Wproj[:, g * GD : (g + 1) * GD].T.reshape(NH, HD, D)  # [head, d, m]
        wp2 = np.ascontiguousarray(
            np.stack(
                [
                    np.concatenate([wsl[0], wsl[1]], axis=0),
                    np.concatenate([wsl[2], wsl[3]], axis=0),
                ],
                axis=1,
            ).reshape(P, 2 * D)
        )
        qg8 = np.ascontiguousarray(
            (q_gain[g * NH : (g + 1) * NH] / 8.0).astype(np.float32).reshape(1, NH)
        )
        maps.append(
            {
                "xT": xT,
                "wqkv": wqkv,
                "wp2": wp2,
                "cosn": cosn,
                "sinn": sinn,
                "masks": masks,
                "qg8": qg8,
            }
        )
    return maps


def kernel(x, Wq, Wk, Wv, Wproj, q_gain, _collect=None):
    x = np.asarray(x, dtype=np.float32)
    Wq = np.asarray(Wq, dtype=np.float32)
    Wk = np.asarray(Wk, dtype=np.float32)
    Wv = np.asarray(Wv, dtype=np.float32)
    Wproj = np.asarray(Wproj, dtype=np.float32)
    q_gain = np.asarray(q_gain, dtype=np.float32)

    nc = _get_program()
    maps = _in_maps(x, Wq, Wk, Wv, Wproj, q_gain)
    res = run_bass_kernel_spmd(nc, maps, core_ids=list(range(NC)))
    if _collect is not None:
        _collect.append(res)

    out = np.zeros((B, S, D), dtype=np.float64)
    for core in range(NC):
        b, _ = divmod(core, KV)
        out[b] += res.results[core]["ypt"].T.astype(np.float64)
    return out.astype(np.float32)
